# revision 6
# baseline (speedup 1.0000x reference)
"""RQ-VAE forward pass on 8 TRN2 NeuronCores (pure data parallel over batch).

Matches reference: h = 0.5*enc(z_sem) + 0.5*enc(z_cf); 4-level residual
quantization (cdist argmin + gather); two decoders; MSE losses.

Outputs (total_loss, z_sem_recon, z_cf_recon, codes) like the reference.

Engine discipline (fp32/transpose matmuls only support ONE sync wait):
- every DMA-loaded tensor that feeds the PE gets a dummy "absorber"
  transpose so its DMA tick enters PE engine order ahead of real use;
- each PSUM tag is read by exactly one engine, matched to the engine that
  produces the lhsT of matmuls targeting that tag (tpD=DVE, tpA/o=ACT).
"""
import os

if os.environ.get("JAX_PLATFORMS") == "cpu":
    # the bass kernel executes through the axon PJRT platform
    os.environ["JAX_PLATFORMS"] = ""

import numpy as np
import concourse.bass as bass
import concourse.mybir as mybir
import concourse.tile as tile
from concourse import bacc
from concourse.bass_utils import run_bass_kernel_spmd
from concourse.masks import make_identity

P = 128
DIN = 384
H = 256
L = 4
K = 1024
F32 = mybir.dt.float32
U32 = mybir.dt.uint32
AFT = mybir.ActivationFunctionType
ALU = mybir.AluOpType
AX = mybir.AxisListType


def _nz(a):
    return bool(np.any(np.asarray(a) != 0))


def _none(a):  # not all-ones
    return bool(np.any(np.asarray(a) != 1))


def build_program(nc: bass.Bass, R: int, flags: dict):
    """Trace the per-core program. R = rows per core (multiple of 128)."""
    T = R // P
    DC = DIN // P  # 3 chunks of the input dim
    HC = H // P    # 2 chunks of the hidden dim

    # ---------------- DRAM tensors ----------------
    zs_d = nc.dram_tensor("zs", [R, DIN], F32, kind="ExternalInput").ap()
    zc_d = nc.dram_tensor("zc", [R, DIN], F32, kind="ExternalInput").ap()
    w1e_d = nc.dram_tensor("w1e", [DIN, H], F32, kind="ExternalInput").ap()
    w2e_d = nc.dram_tensor("w2e", [H, H], F32, kind="ExternalInput").ap()
    cbt2_d = nc.dram_tensor("cbt2", [H, L, K], F32, kind="ExternalInput").ap()
    csqn_d = nc.dram_tensor("csqn", [L, K], F32, kind="ExternalInput").ap()
    cb_d = [nc.dram_tensor(f"cb{l}", [K, H], F32, kind="ExternalInput").ap()
            for l in range(L)]
    ws1_d = nc.dram_tensor("ws1", [H, H], F32, kind="ExternalInput").ap()
    ws2_d = nc.dram_tensor("ws2", [H, DIN], F32, kind="ExternalInput").ap()
    wc1_d = nc.dram_tensor("wc1", [H, H], F32, kind="ExternalInput").ap()
    wc2_d = nc.dram_tensor("wc2", [H, DIN], F32, kind="ExternalInput").ap()

    # optional (nonzero / non-unit) parameter tensors
    opt = {}
    for name, shape, used in [
        ("b1e", [H], flags["b1e"]), ("g1e", [H], flags["g1e"]),
        ("be1e", [H], flags["be1e"]), ("b2e", [H], flags["b2e"]),
        ("g2e", [H], flags["g2e"]), ("be2e", [H], flags["be2e"]),
        ("bs1", [H], flags["bs1"]), ("gs1", [H], flags["gs1"]),
        ("bes1", [H], flags["bes1"]), ("bs2", [DIN], flags["bs2"]),
        ("bc1", [H], flags["bc1"]), ("gc1", [H], flags["gc1"]),
        ("bec1", [H], flags["bec1"]), ("bc2", [DIN], flags["bc2"]),
    ]:
        if used:
            opt[name] = nc.dram_tensor(name, shape, F32, kind="ExternalInput").ap()

    zsr_d = nc.dram_tensor("zsr", [R, DIN], F32, kind="ExternalOutput").ap()
    zcr_d = nc.dram_tensor("zcr", [R, DIN], F32, kind="ExternalOutput").ap()
    codes_d = nc.dram_tensor("codes", [R, L], U32, kind="ExternalOutput").ap()
    part_d = nc.dram_tensor("part", [P, 8], F32, kind="ExternalOutput").ap()

    def bcast(dram_ap, n):
        """AP that replicates a [n] DRAM vector across P partitions."""
        return bass.AP(tensor=dram_ap.tensor, offset=dram_ap.offset,
                       ap=[[0, P], [1, n]])

    with tile.TileContext(nc) as tc:
        with (
            tc.tile_pool(name="const", bufs=1) as cp,
            tc.tile_pool(name="work", bufs=3) as wp,
            tc.tile_pool(name="psDm", bufs=1, space="PSUM") as psDm,
            tc.tile_pool(name="psD", bufs=2, space="PSUM") as psD,
            tc.tile_pool(name="psA", bufs=2, space="PSUM") as psA,
            tc.tile_pool(name="psS", bufs=1, space="PSUM") as psS,
            tc.tile_pool(name="psO", bufs=1, space="PSUM") as psO,
        ):
            # ---------------- constants ----------------
            ident = cp.tile([P, P], F32)
            make_identity(nc, ident)

            def absorb(src_ap):
                d = psDm.tile([P, P], F32, tag="dummy", name="dummy")
                nc.tensor.transpose(d, src_ap, ident)

            absorb(ident)  # prime PE on the identity's Pool sem

            w1e = cp.tile([P, DC, H], F32)
            nc.sync.dma_start(w1e, w1e_d.rearrange("(c p) h -> p c h", p=P))
            w2e = cp.tile([P, HC, H], F32)
            nc.sync.dma_start(w2e, w2e_d.rearrange("(c p) h -> p c h", p=P))
            cbt2 = cp.tile([P, HC, L, K], F32)
            nc.sync.dma_start(cbt2, cbt2_d.rearrange("(c p) l k -> p c l k", p=P))
            csqn = cp.tile([P, L, K], F32)
            nc.sync.dma_start(
                csqn, bass.AP(tensor=csqn_d.tensor, offset=csqn_d.offset,
                              ap=[[0, P], [K, L], [1, K]]))
            wd1 = {}
            wd2 = {}
            for d_, (wa, wb) in {"s": (ws1_d, ws2_d), "c": (wc1_d, wc2_d)}.items():
                wd1[d_] = cp.tile([P, HC, H], F32, name="wd1" + d_)
                nc.sync.dma_start(wd1[d_], wa.rearrange("(c p) h -> p c h", p=P))
                wd2[d_] = cp.tile([P, HC, DIN], F32, name="wd2" + d_)
                nc.sync.dma_start(wd2[d_], wb.rearrange("(c p) h -> p c h", p=P))
            for t_ in (w1e, w2e, wd1["s"], wd2["s"], wd1["c"], wd2["c"]):
                absorb(t_[:, 0, 0:P])
            absorb(cbt2[:, 0, 0, 0:P])

            ob = {}
            for name, ap_ in opt.items():
                n = ap_.shape[0]
                if name in ("g1e", "be1e", "gs1", "bes1", "gc1", "bec1"):
                    # per-partition layout [P, HC] for post-transpose ACT
                    t_ = cp.tile([P, HC], F32, name=name + "_pp")
                    nc.sync.dma_start(t_, ap_.rearrange("(c p) -> p c", p=P))
                else:
                    t_ = cp.tile([P, n], F32, name=name + "_bc")
                    nc.sync.dma_start(t_, bcast(ap_, n))
                ob[name] = t_

            eps = cp.tile([P, 1], F32)
            nc.vector.memset(eps, 1e-5)
            part_sb = cp.tile([P, 8], F32)
            nc.vector.memset(part_sb, 0.0)
            scrap = cp.tile([P, DIN], F32)
            codes_all = cp.tile([P, T, L], U32)

            # ---------------- helpers ----------------
            def layernorm_stats(v_sb):
                """mean/var of SBUF tile -> (rs, nm) per-partition tiles."""
                bn6 = wp.tile([P, 6], F32, tag="bn6")
                nc.vector.bn_stats(bn6, v_sb)
                bn2 = wp.tile([P, 2], F32, tag="bn2")
                nc.vector.bn_aggr(bn2, bn6)
                std = wp.tile([P, 1], F32, tag="std")
                nc.scalar.activation(std, bn2[:, 1:2], AFT.Sqrt, bias=eps[:, 0:1])
                rs = wp.tile([P, 1], F32, tag="rs")
                nc.vector.reciprocal(rs, std)
                nm = wp.tile([P, 1], F32, tag="nm")
                nc.vector.tensor_scalar(nm, bn2[:, 0:1], rs[:, 0:1], -1.0,
                                        op0=ALU.mult, op1=ALU.mult)
                return rs, nm

            def gelu_transposed(u_sb, g1, be1, tag):
                """transpose u on PE, gelu(+affine) from PSUM -> [P, HC, P]."""
                uT_ps = psA.tile([P, HC, P], F32, tag="tpA")
                for c in range(HC):
                    nc.tensor.transpose(uT_ps[:, c], u_sb[:, c * P:(c + 1) * P],
                                        ident)
                hgT = wp.tile([P, HC, P], F32, tag=tag)
                if g1 is not None or be1 is not None:
                    for c in range(HC):
                        kw = {}
                        if g1 is not None:
                            kw["scale"] = g1[:, c:c + 1]
                        if be1 is not None:
                            kw["bias"] = be1[:, c:c + 1]
                        nc.scalar.activation(hgT[:, c], uT_ps[:, c], AFT.Gelu, **kw)
                else:
                    nc.scalar.activation(hgT, uT_ps, AFT.Gelu)
                return hgT

            # ---------------- main loop ----------------
            for t in range(T):
                rows = slice(t * P, (t + 1) * P)
                x_in = {}
                u_out = {}
                for nm_, zd in (("s", zs_d), ("c", zc_d)):
                    x = wp.tile([P, DIN], F32, tag="x" + nm_)
                    nc.sync.dma_start(x, zd[rows])
                    x_in[nm_] = x
                    absorb(x[:, 0:P])  # put the x DMA tick into PE order
                    # -------- encoder stage 1 --------
                    xT_ps = psD.tile([P, DC, P], F32, tag="tpD")
                    for c in range(DC):
                        nc.tensor.transpose(xT_ps[:, c], x[:, c * P:(c + 1) * P],
                                            ident)
                    xT = wp.tile([P, DC, P], F32, tag="xT")
                    nc.vector.tensor_copy(xT, xT_ps)
                    h1 = psD.tile([P, H], F32, tag="tpD")
                    for c in range(DC):
                        nc.tensor.matmul(h1, lhsT=xT[:, c], rhs=w1e[:, c],
                                         start=(c == 0), stop=(c == DC - 1))
                    v = wp.tile([P, H], F32, tag="ve")
                    if "b1e" in ob:
                        nc.vector.tensor_tensor(v, h1, ob["b1e"], op=ALU.add)
                    else:
                        nc.vector.tensor_copy(v, h1)
                    rs, nm = layernorm_stats(v)
                    u = wp.tile([P, H], F32, tag="ue")
                    nc.scalar.activation(u, v, AFT.Identity,
                                         bias=nm[:, 0:1], scale=rs[:, 0:1])
                    hgT = gelu_transposed(u, ob.get("g1e"), ob.get("be1e"),
                                          "hgT" + nm_)
                    # -------- encoder stage 2 --------
                    h2 = psA.tile([P, H], F32, tag="tpA")
                    for c in range(HC):
                        nc.tensor.matmul(h2, lhsT=hgT[:, c], rhs=w2e[:, c],
                                         start=(c == 0), stop=(c == HC - 1))
                    v2 = wp.tile([P, H], F32, tag="v2e")
                    nc.scalar.copy(v2, h2)  # ACT: keeps tpA single-reader
                    if "b2e" in ob:
                        v2b = wp.tile([P, H], F32, tag="v2b")
                        nc.vector.tensor_tensor(v2b, v2, ob["b2e"], op=ALU.add)
                        v2 = v2b
                    rs, nm = layernorm_stats(v2)
                    u2 = wp.tile([P, H], F32, tag="uo" + nm_)
                    nc.scalar.activation(u2, v2, AFT.Identity,
                                         bias=nm[:, 0:1], scale=rs[:, 0:1])
                    u_out[nm_] = u2

                hj = wp.tile([P, H], F32, tag="hj")
                nc.vector.tensor_tensor(hj, u_out["s"], u_out["c"], op=ALU.add)
                if "g2e" in ob:
                    nc.vector.tensor_tensor(hj, hj, ob["g2e"], op=ALU.mult)
                nc.vector.tensor_scalar(hj, hj, 0.5, None, op0=ALU.mult)
                if "be2e" in ob:
                    nc.vector.tensor_tensor(hj, hj, ob["be2e"], op=ALU.add)

                # -------- residual quantization --------
                r = wp.tile([P, H], F32, tag="r")
                for l in range(L):
                    src = hj if l == 0 else r
                    rT_ps = psD.tile([P, HC, P], F32, tag="tpD")
                    for c in range(HC):
                        nc.tensor.transpose(rT_ps[:, c], src[:, c * P:(c + 1) * P],
                                            ident)
                    rT = wp.tile([P, HC, P], F32, tag="rT")
                    nc.vector.tensor_copy(rT, rT_ps)
                    sc_ps = psS.tile([P, K], F32, tag="sc")
                    for h_ in range(2):
                        cols = slice(h_ * 512, (h_ + 1) * 512)
                        for c in range(HC):
                            nc.tensor.matmul(sc_ps[:, cols], lhsT=rT[:, c],
                                             rhs=cbt2[:, c, l, cols],
                                             start=(c == 0), stop=(c == HC - 1))
                    scs = wp.tile([P, K], F32, tag="scs")
                    nc.vector.tensor_tensor(scs, sc_ps, csqn[:, l], op=ALU.add)
                    rmax = wp.tile([P, 1], F32, tag="rmax")
                    nc.vector.reduce_max(rmax, scs, axis=AX.X)
                    idx = wp.tile([P, 8], U32, tag="idx")
                    nc.vector.max_index(idx, rmax[:, 0:1].to_broadcast([P, 8]), scs)
                    nc.vector.tensor_copy(codes_all[:, t, l:l + 1], idx[:, 0:1])
                    q = wp.tile([P, H], F32, tag="q")
                    nc.gpsimd.indirect_dma_start(
                        out=q, out_offset=None, in_=cb_d[l],
                        in_offset=bass.IndirectOffsetOnAxis(ap=idx[:, 0:1], axis=0))
                    nc.vector.tensor_tensor(r, src, q, op=ALU.subtract)
                    rsq = wp.tile([P, 1], F32, tag="rsq")
                    nc.scalar.activation(scrap[:, :H], r, AFT.Square,
                                         accum_out=rsq)
                    nc.vector.tensor_tensor(part_sb[:, l:l + 1], part_sb[:, l:l + 1],
                                            rsq, op=ALU.add)

                qz = wp.tile([P, H], F32, tag="qz")
                nc.vector.tensor_tensor(qz, hj, r, op=ALU.subtract)

                qzT_ps = psD.tile([P, HC, P], F32, tag="tpD")
                for c in range(HC):
                    nc.tensor.transpose(qzT_ps[:, c], qz[:, c * P:(c + 1) * P], ident)
                qzT = wp.tile([P, HC, P], F32, tag="qzT")
                nc.vector.tensor_copy(qzT, qzT_ps)

                # -------- decoders --------
                for nm_, zr_d, acc_col in (("s", zsr_d, 4), ("c", zcr_d, 5)):
                    g1d_ps = psD.tile([P, H], F32, tag="tpD")
                    for c in range(HC):
                        nc.tensor.matmul(g1d_ps, lhsT=qzT[:, c], rhs=wd1[nm_][:, c],
                                         start=(c == 0), stop=(c == HC - 1))
                    vd = wp.tile([P, H], F32, tag="vd")
                    if "b" + nm_ + "1" in ob:
                        nc.vector.tensor_tensor(vd, g1d_ps, ob["b" + nm_ + "1"],
                                                op=ALU.add)
                    else:
                        nc.vector.tensor_copy(vd, g1d_ps)
                    rs, nm2 = layernorm_stats(vd)
                    ud = wp.tile([P, H], F32, tag="ud")
                    nc.scalar.activation(ud, vd, AFT.Identity,
                                         bias=nm2[:, 0:1], scale=rs[:, 0:1])
                    hdT = gelu_transposed(ud, ob.get("g" + nm_ + "1"),
                                          ob.get("be" + nm_ + "1"), "hdT")
                    o_ps = psO.tile([P, DIN], F32, tag="o")
                    for c in range(HC):
                        nc.tensor.matmul(o_ps, lhsT=hdT[:, c], rhs=wd2[nm_][:, c],
                                         start=(c == 0), stop=(c == HC - 1))
                    recon = wp.tile([P, DIN], F32, tag="recon" + nm_)
                    nc.scalar.copy(recon, o_ps)  # ACT: keeps psO single-reader
                    if "b" + nm_ + "2" in ob:
                        reconb = wp.tile([P, DIN], F32, tag="reconb" + nm_)
                        nc.vector.tensor_tensor(reconb, recon, ob["b" + nm_ + "2"],
                                                op=ALU.add)
                        recon = reconb
                    nc.sync.dma_start(zr_d[rows], recon)
                    diff = wp.tile([P, DIN], F32, tag="diff")
                    nc.vector.tensor_tensor(diff, recon, x_in[nm_],
                                            op=ALU.subtract)
                    dsq = wp.tile([P, 1], F32, tag="dsq")
                    nc.scalar.activation(scrap, diff, AFT.Square, accum_out=dsq)
                    nc.vector.tensor_tensor(part_sb[:, acc_col:acc_col + 1],
                                            part_sb[:, acc_col:acc_col + 1],
                                            dsq, op=ALU.add)

            nc.sync.dma_start(codes_d.rearrange("(t p) l -> p t l", p=P), codes_all)
            nc.sync.dma_start(part_d, part_sb)
    return nc


def _prep(z_sem, z_cf, enc_params, dec_sem_params, dec_cf_params, codebooks):
    np32 = lambda a: np.ascontiguousarray(np.asarray(a, dtype=np.float32))
    ep = {k: np32(v) for k, v in enc_params.items()}
    dsp = {k: np32(v) for k, v in dec_sem_params.items()}
    dcp = {k: np32(v) for k, v in dec_cf_params.items()}
    cbs = np32(codebooks)

    flags = {
        "b1e": _nz(ep["b1"]), "g1e": _none(ep["g1"]), "be1e": _nz(ep["be1"]),
        "b2e": _nz(ep["b2"]), "g2e": _none(ep["g2"]), "be2e": _nz(ep["be2"]),
        "bs1": _nz(dsp["b1"]), "gs1": _none(dsp["g1"]), "bes1": _nz(dsp["be1"]),
        "bs2": _nz(dsp["b2"]),
        "bc1": _nz(dcp["b1"]), "gc1": _none(dcp["g1"]), "bec1": _nz(dcp["be1"]),
        "bc2": _nz(dcp["b2"]),
    }

    cbt2 = np.ascontiguousarray((2.0 * cbs).transpose(2, 0, 1))  # [H, L, K]
    csqn = np.ascontiguousarray(-np.sum(cbs * cbs, axis=-1))     # [L, K]

    base = {
        "w1e": ep["w1"], "w2e": ep["w2"],
        "cbt2": cbt2, "csqn": csqn,
        "ws1": dsp["w1"], "ws2": dsp["w2"],
        "wc1": dcp["w1"], "wc2": dcp["w2"],
    }
    for l in range(L):
        base[f"cb{l}"] = np.ascontiguousarray(cbs[l])
    optvals = {
        "b1e": ep["b1"], "g1e": ep["g1"], "be1e": ep["be1"],
        "b2e": ep["b2"], "g2e": ep["g2"], "be2e": ep["be2"],
        "bs1": dsp["b1"], "gs1": dsp["g1"], "bes1": dsp["be1"], "bs2": dsp["b2"],
        "bc1": dcp["b1"], "gc1": dcp["g1"], "bec1": dcp["be1"], "bc2": dcp["b2"],
    }
    for name, v in optvals.items():
        if flags[name]:
            base[name] = np32(v)
    return base, flags, np32(z_sem), np32(z_cf)


def kernel(z_sem, z_cf, enc_params, dec_sem_params, dec_cf_params, codebooks,
           n_cores: int = 8):
    base, flags, zs, zc = _prep(z_sem, z_cf, enc_params, dec_sem_params,
                                dec_cf_params, codebooks)
    B = zs.shape[0]
    R = B // n_cores
    assert R % P == 0

    nc = bacc.Bacc("TRN2", target_bir_lowering=False, debug=False,
                   enable_asserts=False, num_devices=n_cores)
    nc = build_program(nc, R, flags)
    nc.compile()

    in_maps = []
    for i in range(n_cores):
        m = dict(base)
        m["zs"] = np.ascontiguousarray(zs[i * R:(i + 1) * R])
        m["zc"] = np.ascontiguousarray(zc[i * R:(i + 1) * R])
        in_maps.append(m)

    trace = bool(int(os.environ.get("KERNEL_TRACE", "0")))
    res = run_bass_kernel_spmd(nc, in_maps, list(range(n_cores)), trace=trace)
    kernel.last_info = {"exec_time_ns": res.exec_time_ns,
                        "profile_json": res.profile_json}

    zsr = np.concatenate([res.results[i]["zsr"] for i in range(n_cores)], axis=0)
    zcr = np.concatenate([res.results[i]["zcr"] for i in range(n_cores)], axis=0)
    codes = np.concatenate(
        [res.results[i]["codes"] for i in range(n_cores)], axis=0
    ).astype(np.int32)
    part = np.stack([res.results[i]["part"] for i in range(n_cores)])  # [C,128,8]

    sums = part.astype(np.float64).sum(axis=(0, 1))  # [8]
    commit = sums[:L].sum() / (B * H) / L
    loss_sem = sums[4] / (B * DIN)
    loss_cf = sums[5] / (B * DIN)
    total = np.float32(0.5 * loss_sem + 0.5 * loss_cf + commit)
    return total, zsr, zcr, codes


# revision 9
# speedup vs baseline: 1.1351x; 1.1351x over previous
"""RQ-VAE forward pass on 8 TRN2 NeuronCores (pure data parallel over batch).

Matches reference: h = 0.5*enc(z_sem) + 0.5*enc(z_cf); 4-level residual
quantization (cdist argmin + gather); two decoders; MSE losses.

Outputs (total_loss, z_sem_recon, z_cf_recon, codes) like the reference.

Engine discipline (fp32/transpose matmuls only support ONE sync wait):
- every DMA-loaded tensor that feeds the PE gets a dummy "absorber"
  transpose so its DMA tick enters PE engine order ahead of real use;
- each PSUM tag is read by exactly one engine, matched to the engine that
  produces the lhsT of matmuls targeting that tag (tpD=DVE, tpA/o=ACT).
"""
import os

if os.environ.get("JAX_PLATFORMS") == "cpu":
    # the bass kernel executes through the axon PJRT platform
    os.environ["JAX_PLATFORMS"] = ""

import numpy as np
import concourse.bass as bass
import concourse.mybir as mybir
import concourse.tile as tile
from concourse import bacc
from concourse.bass_utils import run_bass_kernel_spmd
from concourse.masks import make_identity

P = 128
DIN = 384
H = 256
L = 4
K = 1024
F32 = mybir.dt.float32
U32 = mybir.dt.uint32
AFT = mybir.ActivationFunctionType
ALU = mybir.AluOpType
AX = mybir.AxisListType


def _nz(a):
    return bool(np.any(np.asarray(a) != 0))


def _none(a):  # not all-ones
    return bool(np.any(np.asarray(a) != 1))


def build_program(nc: bass.Bass, R: int, flags: dict, passes: int = 1):
    """Trace the per-core program. R = rows per core (multiple of 128)."""
    T = R // P
    DC = DIN // P  # 3 chunks of the input dim
    HC = H // P    # 2 chunks of the hidden dim

    # ---------------- DRAM tensors ----------------
    zs_d = nc.dram_tensor("zs", [R, DIN], F32, kind="ExternalInput").ap()
    zc_d = nc.dram_tensor("zc", [R, DIN], F32, kind="ExternalInput").ap()
    w1e_d = nc.dram_tensor("w1e", [DIN, H], F32, kind="ExternalInput").ap()
    w2e_d = nc.dram_tensor("w2e", [H, H], F32, kind="ExternalInput").ap()
    cbt2_d = nc.dram_tensor("cbt2", [H, L, K], F32, kind="ExternalInput").ap()
    csqn_d = nc.dram_tensor("csqn", [L, K], F32, kind="ExternalInput").ap()
    cb_d = [nc.dram_tensor(f"cb{l}", [K, H], F32, kind="ExternalInput").ap()
            for l in range(L)]
    ws1_d = nc.dram_tensor("ws1", [H, H], F32, kind="ExternalInput").ap()
    ws2_d = nc.dram_tensor("ws2", [H, DIN], F32, kind="ExternalInput").ap()
    wc1_d = nc.dram_tensor("wc1", [H, H], F32, kind="ExternalInput").ap()
    wc2_d = nc.dram_tensor("wc2", [H, DIN], F32, kind="ExternalInput").ap()

    # optional (nonzero / non-unit) parameter tensors
    opt = {}
    for name, shape, used in [
        ("b1e", [H], flags["b1e"]), ("g1e", [H], flags["g1e"]),
        ("be1e", [H], flags["be1e"]), ("b2e", [H], flags["b2e"]),
        ("g2e", [H], flags["g2e"]), ("be2e", [H], flags["be2e"]),
        ("bs1", [H], flags["bs1"]), ("gs1", [H], flags["gs1"]),
        ("bes1", [H], flags["bes1"]), ("bs2", [DIN], flags["bs2"]),
        ("bc1", [H], flags["bc1"]), ("gc1", [H], flags["gc1"]),
        ("bec1", [H], flags["bec1"]), ("bc2", [DIN], flags["bc2"]),
    ]:
        if used:
            opt[name] = nc.dram_tensor(name, shape, F32, kind="ExternalInput").ap()

    zsr_d = nc.dram_tensor("zsr", [R, DIN], F32, kind="ExternalOutput").ap()
    zcr_d = nc.dram_tensor("zcr", [R, DIN], F32, kind="ExternalOutput").ap()
    codes_d = nc.dram_tensor("codes", [R, L], U32, kind="ExternalOutput").ap()
    part_d = nc.dram_tensor("part", [P, 8], F32, kind="ExternalOutput").ap()

    def bcast(dram_ap, n):
        """AP that replicates a [n] DRAM vector across P partitions."""
        return bass.AP(tensor=dram_ap.tensor, offset=dram_ap.offset,
                       ap=[[0, P], [1, n]])

    with tile.TileContext(nc) as tc:
        with (
            tc.tile_pool(name="const", bufs=1) as cp,
            tc.tile_pool(name="work", bufs=3) as wp,
            tc.tile_pool(name="psDm", bufs=1, space="PSUM") as psDm,
            tc.tile_pool(name="psD", bufs=2, space="PSUM") as psD,
            tc.tile_pool(name="psA", bufs=2, space="PSUM") as psA,
            tc.tile_pool(name="psS", bufs=1, space="PSUM") as psS,
            tc.tile_pool(name="psO", bufs=1, space="PSUM") as psO,
        ):
            # ---------------- constants ----------------
            ident = cp.tile([P, P], F32)
            make_identity(nc, ident)

            def absorb(src_ap):
                d = psDm.tile([P, P], F32, tag="dummy", name="dummy")
                nc.tensor.transpose(d, src_ap, ident)

            absorb(ident)  # prime PE on the identity's Pool sem

            w1e = cp.tile([P, DC, H], F32)
            nc.sync.dma_start(w1e, w1e_d.rearrange("(c p) h -> p c h", p=P))
            w2e = cp.tile([P, HC, H], F32)
            nc.sync.dma_start(w2e, w2e_d.rearrange("(c p) h -> p c h", p=P))
            cbt2 = cp.tile([P, HC, L, K], F32)
            nc.sync.dma_start(cbt2, cbt2_d.rearrange("(c p) l k -> p c l k", p=P))
            csqn = cp.tile([P, L, K], F32)
            nc.sync.dma_start(
                csqn, bass.AP(tensor=csqn_d.tensor, offset=csqn_d.offset,
                              ap=[[0, P], [K, L], [1, K]]))
            wd1 = {}
            wd2 = {}
            for d_, (wa, wb) in {"s": (ws1_d, ws2_d), "c": (wc1_d, wc2_d)}.items():
                wd1[d_] = cp.tile([P, HC, H], F32, name="wd1" + d_)
                nc.sync.dma_start(wd1[d_], wa.rearrange("(c p) h -> p c h", p=P))
                wd2[d_] = cp.tile([P, HC, DIN], F32, name="wd2" + d_)
                nc.sync.dma_start(wd2[d_], wb.rearrange("(c p) h -> p c h", p=P))
            for t_ in (w1e, w2e, wd1["s"], wd2["s"], wd1["c"], wd2["c"]):
                absorb(t_[:, 0, 0:P])
            absorb(cbt2[:, 0, 0, 0:P])

            ob = {}
            for name, ap_ in opt.items():
                n = ap_.shape[0]
                if name in ("g1e", "be1e", "gs1", "bes1", "gc1", "bec1"):
                    # per-partition layout [P, HC] for post-transpose ACT
                    t_ = cp.tile([P, HC], F32, name=name + "_pp")
                    nc.sync.dma_start(t_, ap_.rearrange("(c p) -> p c", p=P))
                else:
                    t_ = cp.tile([P, n], F32, name=name + "_bc")
                    nc.sync.dma_start(t_, bcast(ap_, n))
                ob[name] = t_

            part_sb = cp.tile([P, 8], F32)
            nc.vector.memset(part_sb, 0.0)
            scrap = cp.tile([P, DIN], F32)
            codes_all = cp.tile([P, T, L], U32)

            # ---------------- helpers ----------------
            def layernorm_stats(v_sb):
                """mean/var of SBUF tile -> (rs, nm) per-partition tiles."""
                bn6 = wp.tile([P, 6], F32, tag="bn6")
                nc.vector.bn_stats(bn6, v_sb)
                bn2 = wp.tile([P, 2], F32, tag="bn2")
                nc.vector.bn_aggr(bn2, bn6)
                rs = wp.tile([P, 1], F32, tag="rs")
                nc.vector.tensor_scalar(rs, bn2[:, 1:2], 1e-5, -0.5,
                                        op0=ALU.add, op1=ALU.pow)
                nm = wp.tile([P, 1], F32, tag="nm")
                nc.vector.tensor_scalar(nm, bn2[:, 0:1], rs[:, 0:1], -1.0,
                                        op0=ALU.mult, op1=ALU.mult)
                return rs, nm

            def gelu_transposed(u_sb, g1, be1, tag):
                """transpose u on PE, gelu(+affine) from PSUM -> [P, HC, P]."""
                uT_ps = psA.tile([P, HC, P], F32, tag="tpA")
                for c in range(HC):
                    nc.tensor.transpose(uT_ps[:, c], u_sb[:, c * P:(c + 1) * P],
                                        ident)
                hgT = wp.tile([P, HC, P], F32, tag=tag)
                if g1 is not None or be1 is not None:
                    for c in range(HC):
                        kw = {}
                        if g1 is not None:
                            kw["scale"] = g1[:, c:c + 1]
                        if be1 is not None:
                            kw["bias"] = be1[:, c:c + 1]
                        nc.scalar.activation(hgT[:, c], uT_ps[:, c], AFT.Gelu, **kw)
                else:
                    nc.scalar.activation(hgT, uT_ps, AFT.Gelu)
                return hgT

            # ---------------- main loop ----------------
            for t in [tt for _ in range(passes) for tt in range(T)]:
                rows = slice(t * P, (t + 1) * P)
                x_in = {}
                u_out = {}
                for nm_, zd in (("s", zs_d), ("c", zc_d)):
                    x = wp.tile([P, DIN], F32, tag="x" + nm_)
                    nc.sync.dma_start(x, zd[rows])
                    x_in[nm_] = x
                    absorb(x[:, 0:P])  # put the x DMA tick into PE order
                    # -------- encoder stage 1 --------
                    xT_ps = psD.tile([P, DC, P], F32, tag="tpD")
                    for c in range(DC):
                        nc.tensor.transpose(xT_ps[:, c], x[:, c * P:(c + 1) * P],
                                            ident)
                    xT = wp.tile([P, DC, P], F32, tag="xT")
                    nc.vector.tensor_copy(xT, xT_ps)
                    h1 = psD.tile([P, H], F32, tag="tpD")
                    for c in range(DC):
                        nc.tensor.matmul(h1, lhsT=xT[:, c], rhs=w1e[:, c],
                                         start=(c == 0), stop=(c == DC - 1))
                    v = wp.tile([P, H], F32, tag="ve")
                    if "b1e" in ob:
                        nc.vector.tensor_tensor(v, h1, ob["b1e"], op=ALU.add)
                    else:
                        nc.vector.tensor_copy(v, h1)
                    rs, nm = layernorm_stats(v)
                    u = wp.tile([P, H], F32, tag="ue")
                    nc.scalar.activation(u, v, AFT.Identity,
                                         bias=nm[:, 0:1], scale=rs[:, 0:1])
                    hgT = gelu_transposed(u, ob.get("g1e"), ob.get("be1e"),
                                          "hgT" + nm_)
                    # -------- encoder stage 2 --------
                    h2 = psA.tile([P, H], F32, tag="tpA")
                    for c in range(HC):
                        nc.tensor.matmul(h2, lhsT=hgT[:, c], rhs=w2e[:, c],
                                         start=(c == 0), stop=(c == HC - 1))
                    v2 = wp.tile([P, H], F32, tag="v2e")
                    nc.scalar.copy(v2, h2)  # ACT: keeps tpA single-reader
                    if "b2e" in ob:
                        v2b = wp.tile([P, H], F32, tag="v2b")
                        nc.vector.tensor_tensor(v2b, v2, ob["b2e"], op=ALU.add)
                        v2 = v2b
                    rs, nm = layernorm_stats(v2)
                    u2 = wp.tile([P, H], F32, tag="uo" + nm_)
                    nc.scalar.activation(u2, v2, AFT.Identity,
                                         bias=nm[:, 0:1], scale=rs[:, 0:1])
                    u_out[nm_] = u2

                hj = wp.tile([P, H], F32, tag="hj")
                nc.gpsimd.tensor_tensor(hj, u_out["s"], u_out["c"], op=ALU.add)
                if "g2e" in ob:
                    nc.vector.tensor_tensor(hj, hj, ob["g2e"], op=ALU.mult)
                nc.gpsimd.tensor_scalar(hj, hj, 0.5, None, op0=ALU.mult)
                if "be2e" in ob:
                    nc.vector.tensor_tensor(hj, hj, ob["be2e"], op=ALU.add)

                # -------- residual quantization --------
                r = wp.tile([P, H], F32, tag="r")
                for l in range(L):
                    src = hj if l == 0 else r
                    rT_ps = psD.tile([P, HC, P], F32, tag="tpD")
                    for c in range(HC):
                        nc.tensor.transpose(rT_ps[:, c], src[:, c * P:(c + 1) * P],
                                            ident)
                    rT = wp.tile([P, HC, P], F32, tag="rT")
                    nc.vector.tensor_copy(rT, rT_ps)
                    sc_ps = psS.tile([P, K], F32, tag="sc")
                    for h_ in range(2):
                        cols = slice(h_ * 512, (h_ + 1) * 512)
                        for c in range(HC):
                            nc.tensor.matmul(sc_ps[:, cols], lhsT=rT[:, c],
                                             rhs=cbt2[:, c, l, cols],
                                             start=(c == 0), stop=(c == HC - 1))
                    scs = wp.tile([P, K], F32, tag="scs")
                    nc.vector.tensor_tensor(scs, sc_ps, csqn[:, l], op=ALU.add)
                    rmax = wp.tile([P, 1], F32, tag="rmax")
                    nc.vector.reduce_max(rmax, scs, axis=AX.X)
                    idx = wp.tile([P, 8], U32, tag="idx")
                    nc.vector.max_index(idx, rmax[:, 0:1].to_broadcast([P, 8]), scs)
                    nc.vector.tensor_copy(codes_all[:, t, l:l + 1], idx[:, 0:1])
                    q = wp.tile([P, H], F32, tag="q")
                    nc.gpsimd.indirect_dma_start(
                        out=q, out_offset=None, in_=cb_d[l],
                        in_offset=bass.IndirectOffsetOnAxis(ap=idx[:, 0:1], axis=0))
                    nc.vector.tensor_tensor(r, src, q, op=ALU.subtract)
                    rsq = wp.tile([P, 1], F32, tag="rsq")
                    nc.scalar.activation(scrap[:, :H], r, AFT.Square,
                                         accum_out=rsq)
                    nc.vector.tensor_tensor(part_sb[:, l:l + 1], part_sb[:, l:l + 1],
                                            rsq, op=ALU.add)

                qz = wp.tile([P, H], F32, tag="qz")
                nc.vector.tensor_tensor(qz, hj, r, op=ALU.subtract)

                qzT_ps = psD.tile([P, HC, P], F32, tag="tpD")
                for c in range(HC):
                    nc.tensor.transpose(qzT_ps[:, c], qz[:, c * P:(c + 1) * P], ident)
                qzT = wp.tile([P, HC, P], F32, tag="qzT")
                nc.vector.tensor_copy(qzT, qzT_ps)

                # -------- decoders --------
                for nm_, zr_d, acc_col in (("s", zsr_d, 4), ("c", zcr_d, 5)):
                    g1d_ps = psD.tile([P, H], F32, tag="tpD")
                    for c in range(HC):
                        nc.tensor.matmul(g1d_ps, lhsT=qzT[:, c], rhs=wd1[nm_][:, c],
                                         start=(c == 0), stop=(c == HC - 1))
                    vd = wp.tile([P, H], F32, tag="vd")
                    if "b" + nm_ + "1" in ob:
                        nc.vector.tensor_tensor(vd, g1d_ps, ob["b" + nm_ + "1"],
                                                op=ALU.add)
                    else:
                        nc.vector.tensor_copy(vd, g1d_ps)
                    rs, nm2 = layernorm_stats(vd)
                    ud = wp.tile([P, H], F32, tag="ud")
                    nc.scalar.activation(ud, vd, AFT.Identity,
                                         bias=nm2[:, 0:1], scale=rs[:, 0:1])
                    hdT = gelu_transposed(ud, ob.get("g" + nm_ + "1"),
                                          ob.get("be" + nm_ + "1"), "hdT")
                    o_ps = psO.tile([P, DIN], F32, tag="o")
                    for c in range(HC):
                        nc.tensor.matmul(o_ps, lhsT=hdT[:, c], rhs=wd2[nm_][:, c],
                                         start=(c == 0), stop=(c == HC - 1))
                    recon = wp.tile([P, DIN], F32, tag="recon" + nm_)
                    nc.scalar.copy(recon, o_ps)  # ACT: keeps psO single-reader
                    if "b" + nm_ + "2" in ob:
                        reconb = wp.tile([P, DIN], F32, tag="reconb" + nm_)
                        nc.vector.tensor_tensor(reconb, recon, ob["b" + nm_ + "2"],
                                                op=ALU.add)
                        recon = reconb
                    nc.sync.dma_start(zr_d[rows], recon)
                    diff = wp.tile([P, DIN], F32, tag="diff")
                    nc.gpsimd.tensor_tensor(diff, recon, x_in[nm_],
                                            op=ALU.subtract)
                    dsq = wp.tile([P, 1], F32, tag="dsq")
                    nc.scalar.activation(scrap, diff, AFT.Square, accum_out=dsq)
                    nc.vector.tensor_tensor(part_sb[:, acc_col:acc_col + 1],
                                            part_sb[:, acc_col:acc_col + 1],
                                            dsq, op=ALU.add)

            nc.sync.dma_start(codes_d.rearrange("(t p) l -> p t l", p=P), codes_all)
            nc.sync.dma_start(part_d, part_sb)
    return nc


def _prep(z_sem, z_cf, enc_params, dec_sem_params, dec_cf_params, codebooks):
    np32 = lambda a: np.ascontiguousarray(np.asarray(a, dtype=np.float32))
    ep = {k: np32(v) for k, v in enc_params.items()}
    dsp = {k: np32(v) for k, v in dec_sem_params.items()}
    dcp = {k: np32(v) for k, v in dec_cf_params.items()}
    cbs = np32(codebooks)

    flags = {
        "b1e": _nz(ep["b1"]), "g1e": _none(ep["g1"]), "be1e": _nz(ep["be1"]),
        "b2e": _nz(ep["b2"]), "g2e": _none(ep["g2"]), "be2e": _nz(ep["be2"]),
        "bs1": _nz(dsp["b1"]), "gs1": _none(dsp["g1"]), "bes1": _nz(dsp["be1"]),
        "bs2": _nz(dsp["b2"]),
        "bc1": _nz(dcp["b1"]), "gc1": _none(dcp["g1"]), "bec1": _nz(dcp["be1"]),
        "bc2": _nz(dcp["b2"]),
    }

    cbt2 = np.ascontiguousarray((2.0 * cbs).transpose(2, 0, 1))  # [H, L, K]
    csqn = np.ascontiguousarray(-np.sum(cbs * cbs, axis=-1))     # [L, K]

    base = {
        "w1e": ep["w1"], "w2e": ep["w2"],
        "cbt2": cbt2, "csqn": csqn,
        "ws1": dsp["w1"], "ws2": dsp["w2"],
        "wc1": dcp["w1"], "wc2": dcp["w2"],
    }
    for l in range(L):
        base[f"cb{l}"] = np.ascontiguousarray(cbs[l])
    optvals = {
        "b1e": ep["b1"], "g1e": ep["g1"], "be1e": ep["be1"],
        "b2e": ep["b2"], "g2e": ep["g2"], "be2e": ep["be2"],
        "bs1": dsp["b1"], "gs1": dsp["g1"], "bes1": dsp["be1"], "bs2": dsp["b2"],
        "bc1": dcp["b1"], "gc1": dcp["g1"], "bec1": dcp["be1"], "bc2": dcp["b2"],
    }
    for name, v in optvals.items():
        if flags[name]:
            base[name] = np32(v)
    return base, flags, np32(z_sem), np32(z_cf)


def kernel(z_sem, z_cf, enc_params, dec_sem_params, dec_cf_params, codebooks,
           n_cores: int = 8):
    base, flags, zs, zc = _prep(z_sem, z_cf, enc_params, dec_sem_params,
                                dec_cf_params, codebooks)
    B = zs.shape[0]
    R = B // n_cores
    assert R % P == 0

    nc = bacc.Bacc("TRN2", target_bir_lowering=False, debug=False,
                   enable_asserts=False, num_devices=n_cores)
    nc = build_program(nc, R, flags)
    nc.compile()

    in_maps = []
    for i in range(n_cores):
        m = dict(base)
        m["zs"] = np.ascontiguousarray(zs[i * R:(i + 1) * R])
        m["zc"] = np.ascontiguousarray(zc[i * R:(i + 1) * R])
        in_maps.append(m)

    trace = bool(int(os.environ.get("KERNEL_TRACE", "0")))
    res = run_bass_kernel_spmd(nc, in_maps, list(range(n_cores)), trace=trace)
    kernel.last_info = {"exec_time_ns": res.exec_time_ns,
                        "profile_json": res.profile_json}

    zsr = np.concatenate([res.results[i]["zsr"] for i in range(n_cores)], axis=0)
    zcr = np.concatenate([res.results[i]["zcr"] for i in range(n_cores)], axis=0)
    codes = np.concatenate(
        [res.results[i]["codes"] for i in range(n_cores)], axis=0
    ).astype(np.int32)
    part = np.stack([res.results[i]["part"] for i in range(n_cores)])  # [C,128,8]

    sums = part.astype(np.float64).sum(axis=(0, 1))  # [8]
    commit = sums[:L].sum() / (B * H) / L
    loss_sem = sums[4] / (B * DIN)
    loss_cf = sums[5] / (B * DIN)
    total = np.float32(0.5 * loss_sem + 0.5 * loss_cf + commit)
    return total, zsr, zcr, codes


# revision 20
# speedup vs baseline: 151.4030x; 133.3840x over previous
"""RQ-VAE forward pass on 8 TRN2 NeuronCores (pure data parallel over batch).

Matches reference: h = 0.5*enc(z_sem) + 0.5*enc(z_cf); 4-level residual
quantization (cdist argmin + gather); two decoders; MSE losses.

Outputs (total_loss, z_sem_recon, z_cf_recon, codes) like the reference.

Performance structure:
- NS row-tiles ("streams") are software-pipelined; emission is grouped
  stage-by-stage across streams so the in-order engine sequencers always
  hold independent ready work while each stream's serial RQ chain
  (matmul -> argmin scan -> gather -> subtract) is in flight.
- Per-stream PSUM pool (1 bank) + shared double-buffered score pool.
- The two encoder inputs are processed as a pair inside one PSUM bank.
- ACT functions: Sqrt (own table set) + the gelu set {Gelu,
  Identity, Copy, Square}; rsqrt = Sqrt + DVE reciprocal for accuracy,
  stage-grouped to minimize activation-table reloads.
"""
import contextlib
import os

if os.environ.get("JAX_PLATFORMS") == "cpu":
    # the bass kernel executes through the axon PJRT platform
    os.environ["JAX_PLATFORMS"] = ""

import numpy as np
import concourse.bass as bass
import concourse.mybir as mybir
import concourse.tile as tile
from concourse import bacc
from concourse.bass_utils import run_bass_kernel_spmd
from concourse.masks import make_identity

P = 128
DIN = 384
H = 256
L = 4
K = 1024
F32 = mybir.dt.float32
U32 = mybir.dt.uint32
AFT = mybir.ActivationFunctionType
ALU = mybir.AluOpType
AX = mybir.AxisListType


def _nz(a):
    return bool(np.any(np.asarray(a) != 0))


def _none(a):  # not all-ones
    return bool(np.any(np.asarray(a) != 1))


def build_program(nc: bass.Bass, R: int, flags: dict, passes: int = 1):
    """Trace the per-core program. R = rows per core."""
    T = R // P
    DC = DIN // P  # 3 chunks of the input dim
    HC = H // P    # 2 chunks of the hidden dim

    # ---------------- DRAM tensors ----------------
    zs_d = nc.dram_tensor("zs", [R, DIN], F32, kind="ExternalInput").ap()
    zc_d = nc.dram_tensor("zc", [R, DIN], F32, kind="ExternalInput").ap()
    zsT_d = nc.dram_tensor("zsT", [DIN, R], F32, kind="ExternalInput").ap()
    zcT_d = nc.dram_tensor("zcT", [DIN, R], F32, kind="ExternalInput").ap()
    w1e_d = nc.dram_tensor("w1e", [DIN, H], F32, kind="ExternalInput").ap()
    w2e_d = nc.dram_tensor("w2e", [H, H], F32, kind="ExternalInput").ap()
    cbt2_d = nc.dram_tensor("cbt2", [H, L, K], F32, kind="ExternalInput").ap()
    csqn_d = nc.dram_tensor("csqn", [L, K], F32, kind="ExternalInput").ap()
    cb_d = [nc.dram_tensor(f"cb{l}", [K, H], F32, kind="ExternalInput").ap()
            for l in range(L)]
    ws1_d = nc.dram_tensor("ws1", [H, H], F32, kind="ExternalInput").ap()
    ws2_d = nc.dram_tensor("ws2", [H, DIN], F32, kind="ExternalInput").ap()
    wc1_d = nc.dram_tensor("wc1", [H, H], F32, kind="ExternalInput").ap()
    wc2_d = nc.dram_tensor("wc2", [H, DIN], F32, kind="ExternalInput").ap()

    opt = {}
    for name, shape, used in [
        ("b1e", [H], flags["b1e"]), ("g1e", [H], flags["g1e"]),
        ("be1e", [H], flags["be1e"]), ("b2e", [H], flags["b2e"]),
        ("g2e", [H], flags["g2e"]), ("be2e", [H], flags["be2e"]),
        ("bs1", [H], flags["bs1"]), ("gs1", [H], flags["gs1"]),
        ("bes1", [H], flags["bes1"]), ("bs2", [DIN], flags["bs2"]),
        ("bc1", [H], flags["bc1"]), ("gc1", [H], flags["gc1"]),
        ("bec1", [H], flags["bec1"]), ("bc2", [DIN], flags["bc2"]),
    ]:
        if used:
            opt[name] = nc.dram_tensor(name, shape, F32, kind="ExternalInput").ap()

    zsr_d = nc.dram_tensor("zsr", [R, DIN], F32, kind="ExternalOutput").ap()
    zcr_d = nc.dram_tensor("zcr", [R, DIN], F32, kind="ExternalOutput").ap()
    codes_d = nc.dram_tensor("codes", [R, L], U32, kind="ExternalOutput").ap()
    part_d = nc.dram_tensor("part", [P, 8], F32, kind="ExternalOutput").ap()

    def bcast(dram_ap, n):
        return bass.AP(tensor=dram_ap.tensor, offset=dram_ap.offset,
                       ap=[[0, P], [1, n]])

    NS = int(os.environ.get("K_STREAMS", "4"))
    SCB = int(os.environ.get("K_SCBUFS", "2"))
    WPB = int(os.environ.get("K_WP", "1"))
    assert T % NS == 0
    with tile.TileContext(nc) as tc:
        with contextlib.ExitStack() as ctx:
            cp = ctx.enter_context(tc.tile_pool(name="const", bufs=1))
            wp = ctx.enter_context(tc.tile_pool(name="work", bufs=WPB))
            psS = ctx.enter_context(tc.tile_pool(name="psS", bufs=SCB,
                                                 space="PSUM"))
            psR = [ctx.enter_context(
                tc.tile_pool(name=f"psR{i}", bufs=1, space="PSUM"))
                for i in range(NS)]

            # ---------------- constants ----------------
            ident = cp.tile([P, P], F32)
            make_identity(nc, ident)

            w1e = cp.tile([P, DC, H], F32)
            nc.sync.dma_start(w1e, w1e_d.rearrange("(c p) h -> p c h", p=P))
            w2e = cp.tile([P, HC, H], F32)
            nc.sync.dma_start(w2e, w2e_d.rearrange("(c p) h -> p c h", p=P))
            cbt2 = cp.tile([P, HC, L, K], F32)
            nc.sync.dma_start(cbt2, cbt2_d.rearrange("(c p) l k -> p c l k", p=P))
            csqn = cp.tile([P, L, K], F32)
            nc.sync.dma_start(
                csqn, bass.AP(tensor=csqn_d.tensor, offset=csqn_d.offset,
                              ap=[[0, P], [K, L], [1, K]]))
            wd1 = {}
            wd2 = {}
            for d_, (wa, wb) in {"s": (ws1_d, ws2_d), "c": (wc1_d, wc2_d)}.items():
                wd1[d_] = cp.tile([P, HC, H], F32, name="wd1" + d_)
                nc.sync.dma_start(wd1[d_], wa.rearrange("(c p) h -> p c h", p=P))
                wd2[d_] = cp.tile([P, HC, DIN], F32, name="wd2" + d_)
                nc.sync.dma_start(wd2[d_], wb.rearrange("(c p) h -> p c h", p=P))

            ob = {}
            for name, ap_ in opt.items():
                n = ap_.shape[0]
                if name in ("g1e", "be1e", "gs1", "bes1", "gc1", "bec1"):
                    t_ = cp.tile([P, HC], F32, name=name + "_pp")
                    nc.sync.dma_start(t_, ap_.rearrange("(c p) -> p c", p=P))
                else:
                    t_ = cp.tile([P, n], F32, name=name + "_bc")
                    nc.sync.dma_start(t_, bcast(ap_, n))
                ob[name] = t_

            eps = cp.tile([P, 1], F32)
            nc.vector.memset(eps, 1e-5)
            part_acc = []
            scrap = []
            for i in range(NS):
                pa = cp.tile([P, 8], F32, name=f"part_acc{i}")
                nc.vector.memset(pa, 0.0)
                part_acc.append(pa)
                scrap.append(cp.tile([P, DIN], F32, name=f"scrap{i}"))
            codes_all = cp.tile([P, T, L], U32)

            # ---------------- helpers ----------------
            def ln_stats_pair(i, vp):
                """paired [P,2,H] stats -> (rs2 [P,2], nm2 [P,2])."""
                bn12 = wp.tile([P, 2, 6], F32, tag=f"bn12_{i}")
                for g in range(2):
                    nc.vector.bn_stats(bn12[:, g], vp[:, g])
                ag = wp.tile([P, 2, 2], F32, tag=f"ag_{i}")
                for g in range(2):
                    nc.vector.bn_aggr(ag[:, g], bn12[:, g])
                std2 = wp.tile([P, 2], F32, tag=f"std2_{i}")
                nc.scalar.activation(std2, ag[:, :, 1], AFT.Sqrt,
                                     bias=eps[:, 0:1])
                rs2 = wp.tile([P, 2], F32, tag=f"rs2_{i}")
                nc.vector.reciprocal(rs2, std2)
                nm2 = wp.tile([P, 2], F32, tag=f"nm2_{i}")
                nc.vector.tensor_tensor(nm2, ag[:, :, 0], rs2, op=ALU.mult)
                nc.vector.tensor_scalar(nm2, nm2, -1.0, None, op0=ALU.mult)
                return rs2, nm2

            def ln_stats(i, v_ap):
                """single [P,H] stats -> (rs [P,1], nm [P,1])."""
                bn6 = wp.tile([P, 6], F32, tag=f"bn6_{i}")
                nc.vector.bn_stats(bn6, v_ap)
                bn2 = wp.tile([P, 2], F32, tag=f"bn2_{i}")
                nc.vector.bn_aggr(bn2, bn6)
                std = wp.tile([P, 1], F32, tag=f"std_{i}")
                nc.scalar.activation(std, bn2[:, 1:2], AFT.Sqrt,
                                     bias=eps[:, 0:1])
                rs = wp.tile([P, 1], F32, tag=f"rs_{i}")
                nc.vector.reciprocal(rs, std)
                nm = wp.tile([P, 1], F32, tag=f"nm_{i}")
                nc.vector.tensor_scalar(nm, bn2[:, 0:1], rs[:, 0:1], -1.0,
                                        op0=ALU.mult, op1=ALU.mult)
                return rs, nm

            # ---------------- main loop: groups of NS row-tiles ----------------
            group_list = [pp for _ in range(passes) for pp in range(T // NS)]
            for pp in group_list:
                st = [dict(t=NS * pp + i) for i in range(NS)]

                # ---- loads ----
                for i, s in enumerate(st):
                    rows = slice(s["t"] * P, (s["t"] + 1) * P)
                    s["rows"] = rows
                    s["x"] = {}
                    for nm_, zd in (("s", zs_d), ("c", zc_d)):
                        x = wp.tile([P, DIN], F32, tag=f"x{nm_}{i}")
                        nc.sync.dma_start(x, zd[rows])
                        s["x"][nm_] = x

                # ---- enc A: xT loads + paired h1 matmuls ----
                for i, s in enumerate(st):
                    h1p = psR[i].tile([P, 2, H], F32, tag=f"tpR{i}", name="h1p")
                    s["h1p"] = h1p
                    for j, zT in enumerate((zsT_d, zcT_d)):
                        xT = wp.tile([P, DC, P], F32, tag=f"xT{j}{i}")
                        nc.sync.dma_start(
                            xT,
                            zT.rearrange("(c p) r -> p c r", p=P)[:, :, s["rows"]])
                        for c in range(DC):
                            nc.tensor.matmul(h1p[:, j], lhsT=xT[:, c],
                                             rhs=w1e[:, c],
                                             start=(c == 0), stop=(c == DC - 1))

                # ---- enc B: paired stats (rsqrt ACTs grouped) ----
                for i, s in enumerate(st):
                    if "b1e" in ob:
                        vp = wp.tile([P, 2, H], F32, tag=f"v1p{i}")
                        for j in range(2):
                            nc.vector.tensor_tensor(vp[:, j], s["h1p"][:, j],
                                                    ob["b1e"], op=ALU.add)
                        s["v1"] = vp
                    else:
                        s["v1"] = s["h1p"]
                    s["st1"] = ln_stats_pair(i, s["v1"])

                # ---- enc C: normalize + transpose + gelu ----
                for i, s in enumerate(st):
                    rs2, nm2 = s["st1"]
                    up = wp.tile([P, 2, H], F32, tag=f"up{i}")
                    for j in range(2):
                        nc.scalar.activation(up[:, j], s["v1"][:, j], AFT.Identity,
                                             bias=nm2[:, j:j + 1],
                                             scale=rs2[:, j:j + 1])
                    uTp = psR[i].tile([P, 2, HC, P], F32, tag=f"tpR{i}",
                                      name="uTp")
                    for j in range(2):
                        for c in range(HC):
                            nc.tensor.transpose(uTp[:, j, c],
                                                up[:, j, c * P:(c + 1) * P], ident)
                    hgTp = wp.tile([P, 2, HC, P], F32, tag=f"hgTp{i}")
                    if "g1e" in ob or "be1e" in ob:
                        for j in range(2):
                            for c in range(HC):
                                kw = {}
                                if "g1e" in ob:
                                    kw["scale"] = ob["g1e"][:, c:c + 1]
                                if "be1e" in ob:
                                    kw["bias"] = ob["be1e"][:, c:c + 1]
                                nc.scalar.activation(hgTp[:, j, c], uTp[:, j, c],
                                                     AFT.Gelu, **kw)
                    else:
                        nc.scalar.activation(hgTp, uTp, AFT.Gelu)
                    s["hgTp"] = hgTp

                # ---- enc D: paired h2 matmuls ----
                for i, s in enumerate(st):
                    h2p = psR[i].tile([P, 2, H], F32, tag=f"tpR{i}", name="h2p")
                    for j in range(2):
                        for c in range(HC):
                            nc.tensor.matmul(h2p[:, j], lhsT=s["hgTp"][:, j, c],
                                             rhs=w2e[:, c],
                                             start=(c == 0), stop=(c == HC - 1))
                    s["h2p"] = h2p

                # ---- enc E: paired stats ----
                for i, s in enumerate(st):
                    if "b2e" in ob:
                        v2p = wp.tile([P, 2, H], F32, tag=f"v2p{i}")
                        for j in range(2):
                            nc.vector.tensor_tensor(v2p[:, j], s["h2p"][:, j],
                                                    ob["b2e"], op=ALU.add)
                        s["v2"] = v2p
                    else:
                        s["v2"] = s["h2p"]
                    s["st2"] = ln_stats_pair(i, s["v2"])

                # ---- enc F: u2 normalize + joint ----
                for i, s in enumerate(st):
                    rs2, nm2 = s["st2"]
                    u2p = wp.tile([P, 2, H], F32, tag=f"u2p{i}")
                    for j in range(2):
                        nc.scalar.activation(u2p[:, j], s["v2"][:, j], AFT.Identity,
                                             bias=nm2[:, j:j + 1],
                                             scale=rs2[:, j:j + 1])
                    hj = wp.tile([P, H], F32, tag=f"hj{i}")
                    nc.vector.tensor_tensor(hj, u2p[:, 0], u2p[:, 1], op=ALU.add)
                    if "g2e" in ob:
                        nc.vector.tensor_tensor(hj, hj, ob["g2e"], op=ALU.mult)
                    nc.vector.tensor_scalar(hj, hj, 0.5, None, op0=ALU.mult)
                    if "be2e" in ob:
                        nc.vector.tensor_tensor(hj, hj, ob["be2e"], op=ALU.add)
                    s["hj"] = hj
                    s["r"] = wp.tile([P, H], F32, tag=f"r{i}", name="r")

                # ---- residual quantization (levels interleaved over streams) ----
                for l in range(L):
                    for i, s in enumerate(st):
                        src = s["hj"] if l == 0 else s["r"]
                        rT_ps = psR[i].tile([P, HC, P], F32, tag=f"tpR{i}",
                                            name="rT_ps")
                        for c in range(HC):
                            nc.tensor.transpose(rT_ps[:, c],
                                                src[:, c * P:(c + 1) * P], ident)
                        rT = wp.tile([P, HC, P], F32, tag=f"rT{i}")
                        nc.scalar.copy(rT, rT_ps)
                        sc_ps = psS.tile([P, K], F32, tag="sc", name="sc_ps")
                        for h_ in range(2):
                            cols = slice(h_ * 512, (h_ + 1) * 512)
                            for c in range(HC):
                                nc.tensor.matmul(sc_ps[:, cols], lhsT=rT[:, c],
                                                 rhs=cbt2[:, c, l, cols],
                                                 start=(c == 0),
                                                 stop=(c == HC - 1))
                        scs = wp.tile([P, K], F32, tag=f"scs{i}")
                        rx = wp.tile([P, 2], F32, tag=f"rx{i}")
                        for h_ in range(2):
                            cols = slice(h_ * 512, (h_ + 1) * 512)
                            nc.vector.tensor_tensor(scs[:, cols], sc_ps[:, cols],
                                                    csqn[:, l, cols], op=ALU.add)
                            nc.vector.reduce_max(rx[:, h_:h_ + 1],
                                                 scs[:, cols], axis=AX.X)
                        rmax = wp.tile([P, 1], F32, tag=f"rmax{i}")
                        nc.vector.tensor_tensor(rmax, rx[:, 0:1], rx[:, 1:2],
                                                op=ALU.max)
                        idx = wp.tile([P, 8], U32, tag=f"idx{i}")
                        nc.vector.max_index(idx, rmax[:, 0:1].to_broadcast([P, 8]),
                                            scs)
                        nc.vector.tensor_copy(codes_all[:, s["t"], l:l + 1],
                                              idx[:, 0:1])
                        s["idx"] = idx
                    for i, s in enumerate(st):
                        q = wp.tile([P, H], F32, tag=f"q{i}")
                        nc.gpsimd.indirect_dma_start(
                            out=q, out_offset=None, in_=cb_d[l],
                            in_offset=bass.IndirectOffsetOnAxis(
                                ap=s["idx"][:, 0:1], axis=0))
                        src = s["hj"] if l == 0 else s["r"]
                        nc.vector.tensor_tensor(s["r"], src, q, op=ALU.subtract)
                        rsq = wp.tile([P, 1], F32, tag=f"rsq{i}")
                        nc.scalar.activation(scrap[i][:, :H], s["r"], AFT.Square,
                                             accum_out=rsq)
                        nc.vector.tensor_tensor(part_acc[i][:, l:l + 1],
                                                part_acc[i][:, l:l + 1],
                                                rsq, op=ALU.add)

                # ---- quantized ----
                for i, s in enumerate(st):
                    qz = wp.tile([P, H], F32, tag=f"qz{i}")
                    nc.vector.tensor_tensor(qz, s["hj"], s["r"], op=ALU.subtract)
                    qzT_ps = psR[i].tile([P, HC, P], F32, tag=f"tpR{i}",
                                         name="qzT_ps")
                    for c in range(HC):
                        nc.tensor.transpose(qzT_ps[:, c], qz[:, c * P:(c + 1) * P],
                                            ident)
                    qzT = wp.tile([P, HC, P], F32, tag=f"qzT{i}")
                    nc.scalar.copy(qzT, qzT_ps)
                    s["qzT"] = qzT

                # ---- decoders (stage-grouped per decoder over streams) ----
                for nm_, zr_d, acc_col in (("s", zsr_d, 4), ("c", zcr_d, 5)):
                    for i, s in enumerate(st):
                        g1d_ps = psR[i].tile([P, H], F32, tag=f"tpR{i}",
                                             name="g1d_ps")
                        for c in range(HC):
                            nc.tensor.matmul(g1d_ps, lhsT=s["qzT"][:, c],
                                             rhs=wd1[nm_][:, c],
                                             start=(c == 0), stop=(c == HC - 1))
                        if "b" + nm_ + "1" in ob:
                            vd = wp.tile([P, H], F32, tag=f"vd{i}")
                            nc.vector.tensor_tensor(vd, g1d_ps,
                                                    ob["b" + nm_ + "1"], op=ALU.add)
                        else:
                            vd = g1d_ps
                        s["vd"] = vd
                    for i, s in enumerate(st):
                        s["std"] = ln_stats(i, s["vd"])
                    for i, s in enumerate(st):
                        rs, nm2 = s["std"]
                        ud = wp.tile([P, H], F32, tag=f"ud{i}")
                        nc.scalar.activation(ud, s["vd"], AFT.Identity,
                                             bias=nm2[:, 0:1], scale=rs[:, 0:1])
                        udT_ps = psR[i].tile([P, HC, P], F32, tag=f"tpR{i}",
                                             name="udT_ps")
                        for c in range(HC):
                            nc.tensor.transpose(udT_ps[:, c],
                                                ud[:, c * P:(c + 1) * P], ident)
                        hdT = wp.tile([P, HC, P], F32, tag=f"hdT{i}")
                        g1 = ob.get("g" + nm_ + "1")
                        be1 = ob.get("be" + nm_ + "1")
                        if g1 is not None or be1 is not None:
                            for c in range(HC):
                                kw = {}
                                if g1 is not None:
                                    kw["scale"] = g1[:, c:c + 1]
                                if be1 is not None:
                                    kw["bias"] = be1[:, c:c + 1]
                                nc.scalar.activation(hdT[:, c], udT_ps[:, c],
                                                     AFT.Gelu, **kw)
                        else:
                            nc.scalar.activation(hdT, udT_ps, AFT.Gelu)
                        s["hdT"] = hdT
                    for i, s in enumerate(st):
                        o_ps = psR[i].tile([P, DIN], F32, tag=f"tpR{i}",
                                           name="o_ps")
                        for c in range(HC):
                            nc.tensor.matmul(o_ps, lhsT=s["hdT"][:, c],
                                             rhs=wd2[nm_][:, c],
                                             start=(c == 0), stop=(c == HC - 1))
                        recon = wp.tile([P, DIN], F32, tag=f"recon{nm_}{i}")
                        nc.scalar.copy(recon, o_ps)
                        if "b" + nm_ + "2" in ob:
                            reconb = wp.tile([P, DIN], F32, tag=f"reconb{nm_}{i}")
                            nc.vector.tensor_tensor(reconb, recon,
                                                    ob["b" + nm_ + "2"], op=ALU.add)
                            recon = reconb
                        nc.sync.dma_start(zr_d[s["rows"]], recon)
                        diff = wp.tile([P, DIN], F32, tag=f"diff{i}")
                        nc.gpsimd.tensor_tensor(diff, recon, s["x"][nm_],
                                                op=ALU.subtract)
                        dsq = wp.tile([P, 1], F32, tag=f"dsq{i}")
                        nc.scalar.activation(scrap[i], diff, AFT.Square,
                                             accum_out=dsq)
                        nc.vector.tensor_tensor(part_acc[i][:, acc_col:acc_col + 1],
                                                part_acc[i][:, acc_col:acc_col + 1],
                                                dsq, op=ALU.add)

            part_sb = cp.tile([P, 8], F32)
            nc.vector.tensor_tensor(part_sb, part_acc[0], part_acc[1], op=ALU.add)
            for i in range(2, NS):
                nc.vector.tensor_tensor(part_sb, part_sb, part_acc[i], op=ALU.add)
            nc.sync.dma_start(codes_d.rearrange("(t p) l -> p t l", p=P), codes_all)
            nc.sync.dma_start(part_d, part_sb)
    return nc


def _prep(z_sem, z_cf, enc_params, dec_sem_params, dec_cf_params, codebooks):
    np32 = lambda a: np.ascontiguousarray(np.asarray(a, dtype=np.float32))
    ep = {k: np32(v) for k, v in enc_params.items()}
    dsp = {k: np32(v) for k, v in dec_sem_params.items()}
    dcp = {k: np32(v) for k, v in dec_cf_params.items()}
    cbs = np32(codebooks)

    flags = {
        "b1e": _nz(ep["b1"]), "g1e": _none(ep["g1"]), "be1e": _nz(ep["be1"]),
        "b2e": _nz(ep["b2"]), "g2e": _none(ep["g2"]), "be2e": _nz(ep["be2"]),
        "bs1": _nz(dsp["b1"]), "gs1": _none(dsp["g1"]), "bes1": _nz(dsp["be1"]),
        "bs2": _nz(dsp["b2"]),
        "bc1": _nz(dcp["b1"]), "gc1": _none(dcp["g1"]), "bec1": _nz(dcp["be1"]),
        "bc2": _nz(dcp["b2"]),
    }

    cbt2 = np.ascontiguousarray((2.0 * cbs).transpose(2, 0, 1))  # [H, L, K]
    csqn = np.ascontiguousarray(-np.sum(cbs * cbs, axis=-1))     # [L, K]

    base = {
        "w1e": ep["w1"], "w2e": ep["w2"],
        "cbt2": cbt2, "csqn": csqn,
        "ws1": dsp["w1"], "ws2": dsp["w2"],
        "wc1": dcp["w1"], "wc2": dcp["w2"],
    }
    for l in range(L):
        base[f"cb{l}"] = np.ascontiguousarray(cbs[l])
    optvals = {
        "b1e": ep["b1"], "g1e": ep["g1"], "be1e": ep["be1"],
        "b2e": ep["b2"], "g2e": ep["g2"], "be2e": ep["be2"],
        "bs1": dsp["b1"], "gs1": dsp["g1"], "bes1": dsp["be1"], "bs2": dsp["b2"],
        "bc1": dcp["b1"], "gc1": dcp["g1"], "bec1": dcp["be1"], "bc2": dcp["b2"],
    }
    for name, v in optvals.items():
        if flags[name]:
            base[name] = np32(v)
    return base, flags, np32(z_sem), np32(z_cf)


def kernel(z_sem, z_cf, enc_params, dec_sem_params, dec_cf_params, codebooks,
           n_cores: int = 8):
    base, flags, zs, zc = _prep(z_sem, z_cf, enc_params, dec_sem_params,
                                dec_cf_params, codebooks)
    B = zs.shape[0]
    R = B // n_cores

    nc = bacc.Bacc("TRN2", target_bir_lowering=False, debug=False,
                   enable_asserts=False, num_devices=n_cores)
    nc = build_program(nc, R, flags)
    nc.compile()

    in_maps = []
    for i in range(n_cores):
        m = dict(base)
        m["zs"] = np.ascontiguousarray(zs[i * R:(i + 1) * R])
        m["zc"] = np.ascontiguousarray(zc[i * R:(i + 1) * R])
        m["zsT"] = np.ascontiguousarray(m["zs"].T)
        m["zcT"] = np.ascontiguousarray(m["zc"].T)
        in_maps.append(m)

    trace = bool(int(os.environ.get("KERNEL_TRACE", "0")))
    res = run_bass_kernel_spmd(nc, in_maps, list(range(n_cores)), trace=trace)
    kernel.last_info = {"exec_time_ns": res.exec_time_ns,
                        "profile_json": res.profile_json}

    zsr = np.concatenate([res.results[i]["zsr"] for i in range(n_cores)], axis=0)
    zcr = np.concatenate([res.results[i]["zcr"] for i in range(n_cores)], axis=0)
    codes = np.concatenate(
        [res.results[i]["codes"] for i in range(n_cores)], axis=0
    ).astype(np.int32)
    part = np.stack([res.results[i]["part"] for i in range(n_cores)])  # [C,128,8]

    sums = part.astype(np.float64).sum(axis=(0, 1))  # [8]
    commit = sums[:L].sum() / (B * H) / L
    loss_sem = sums[4] / (B * DIN)
    loss_cf = sums[5] / (B * DIN)
    total = np.float32(0.5 * loss_sem + 0.5 * loss_cf + commit)
    return total, zsr, zcr, codes
